# revision 28
# baseline (speedup 1.0000x reference)
"""Trainium2 Bass kernel for nn_BatchConv1d (dynamic per-query conv kernels + banded conv).

Reference computation (per batch b):
    G[i, o]   = (q[b] @ Wk.T + bk)[i, o],  o = c*3 + t   (per-query dynamic kernels)
    bias[i]   = (q[b] @ Wb.T + bb)[i, 0]
    scores[i, j] = sum_{c,t} G[i, c*3+t] * k_pad[b, j+t, c]
    out = scores + bias[:, None] + bias_b

Associativity restructure (2.56x fewer FLOPs than the direct form):
    N[s, j] = sum_{c,t} Wk[3c+t, s] * k_pad[j+t, c]     (stage 1)
    P[t,jj] = sum_c    bk[3c+t]     * k_pad[jj, c]      (bk contribution, 3 rows)
    r[j]    = sum_t P[t, j+t] + bb + bias_b             (rank-1 shift-sum on PE)
    scores  = q @ N' + r'[j],   N'[s,j] = N[s,j] + Wb[0,s]   (Wb folded into N,
              so q @ N' automatically adds the per-query bias)

All transposes / dtype casts / weight re-packing are done host-side (numpy),
so the device only runs the matmul stages plus PSUM evacuations:
  - qT  [512, 1024] bf16  (per batch)      - stage-2 stationary
  - kT  [512, 1026] bf16  (per batch, with zero halo) - stage-1/P moving
  - wk  [4, 128, 1536] bf16 re-packed Wk   - stage-1 stationary
  - bkp [128, 12] bf16: bkp[p, c*3+t] = bk[3*(c*128+p)+t] - P stationary
  - cons [128, 8] f32: cols 0..3 = Wb^T per s-chunk, col 4 = bb + bias_b
Output is written bf16 and upcast to f32 on host.

Sharding: batch data-parallel, 2 batches per core across 8 NeuronCores.
"""
import numpy as np
import ml_dtypes

from concourse import bacc, tile, mybir
from concourse.bass_utils import run_bass_kernel_spmd

BF16 = mybir.dt.bfloat16
F32 = mybir.dt.float32
BF = ml_dtypes.bfloat16
Identity = mybir.ActivationFunctionType.Identity
ADD = mybir.AluOpType.add

B, QL, KL, QS, KS, KW = 16, 1024, 1024, 512, 512, 3
NCORES = 8
B_LOC = B // NCORES      # 2 batches per core
NC_S = QS // 128         # 4 chunks of the s (=QS) contraction dim
NC_C = KS // 128         # 4 chunks of the c (=KS) contraction dim
NI = QL // 128           # 8 i-chunks
NJH = KL // 512          # 2 j-halves
KH = (KL + 2) // 2       # 514: kT half-width (with halo)

_NC_CACHE = {}


def _build():
    nc = bacc.Bacc("TRN2", target_bir_lowering=False, debug=False)
    qt_d = nc.declare_dram_parameter("qT", [B_LOC, QS, QL], BF16, isOutput=False)
    kt_d = nc.declare_dram_parameter("kT", [B_LOC, KS, KL + 2], BF16, isOutput=False)
    wk_d = nc.declare_dram_parameter("wk", [NC_C, 128, KW * QS], BF16, isOutput=False)
    bkp_d = nc.declare_dram_parameter("bkp", [128, NC_C * KW], BF16, isOutput=False)
    cons_d = nc.declare_dram_parameter("cons", [128, 8], F32, isOutput=False)
    out_d = nc.declare_dram_parameter("out", [B_LOC, QL, KL], BF16, isOutput=True)

    with tile.TileContext(nc) as tc:
        with (
            tc.tile_pool(name="const", bufs=1) as cpool,
            tc.tile_pool(name="kq", bufs=2) as kqpool,
            tc.tile_pool(name="nr", bufs=2) as npool,
            tc.tile_pool(name="outp", bufs=3) as opool,
            tc.tile_pool(name="ps_n", bufs=2, space="PSUM") as ps_n,
            tc.tile_pool(name="ps_p", bufs=1, space="PSUM") as ps_p,
            tc.tile_pool(name="ps_r", bufs=1, space="PSUM") as ps_r,
            tc.tile_pool(name="ps_s", bufs=4, space="PSUM") as ps_s,
        ):
            # ---- constants (gpsimd ring) ----
            cons_sb = cpool.tile([128, 8], F32, tag="cons", name="cons")
            nc.gpsimd.dma_start(cons_sb[:], cons_d[:])
            bkp_sb = cpool.tile([128, NC_C * KW], BF16, tag="bkp", name="bkp")
            nc.gpsimd.dma_start(bkp_sb[:], bkp_d[:])
            # all-ones stationary for the rank-1 broadcast matmuls of r
            ones_sb = cpool.tile([1, 128], BF16, tag="ones", name="ones")
            nc.vector.memset(ones_sb[:], 1.0)
            # wk_sb[c][p, t*512 + s] = Wk[3*(c*128+p) + t, s]; per-(c,t) DMAs so
            # the first stage-1 matmuls can start while later slices stream in
            # wk issue split across two otherwise-idle rings to halve the
            # descriptor-issue serialization during the BW-bound warm-up
            wk_sb = [cpool.tile([128, KW * QS], BF16, tag=f"wk{c}", name=f"wk{c}")
                     for c in range(NC_C)]
            for c in range(NC_C):
                eng = nc.gpsimd if c < 2 else nc.scalar
                for t in range(KW):
                    eng.dma_start(
                        wk_sb[c][:, t * QS : (t + 1) * QS],
                        wk_d[c, :, t * QS : (t + 1) * QS],
                    )

            for b in range(B_LOC):
                # ---- loads (sync ring): kT halves first (stage 1), then qT
                kT = [kqpool.tile([128, KL + 2], BF16, tag=f"kT{c}", name=f"kT{c}")
                      for c in range(NC_C)]
                for h in range(2):
                    # batch 0's high halves go on the gpsimd ring so the
                    # critical low halves aren't issue-serialized behind them
                    eng = nc.gpsimd if (b == 0 and h == 1) else nc.sync
                    for c in range(NC_C):
                        eng.dma_start(
                            kT[c][:, h * KH : (h + 1) * KH],
                            kt_d[b, c * 128 : (c + 1) * 128, h * KH : (h + 1) * KH],
                        )
                qT = [kqpool.tile([128, QL], BF16, tag=f"qT{c}", name=f"qT{c}")
                      for c in range(NC_S)]
                for c in range(NC_S):
                    nc.sync.dma_start(qT[c][:], qt_d[b, c * 128 : (c + 1) * 128, :])

                # ---- P[t, jj] = sum_c bk[3c+t] * k_pad[jj, c]   ([3, 1026])
                #      Rows 1,2 are copied to their own tiles so every rank-1
                #      matmul operand sits at base partition 0 (PE rule).
                #      Chunks A+B only need the low kT halves -> emitted first.
                P_sb = npool.tile([3, KL + 2], BF16, tag="psb", name="psb")
                P1T = npool.tile([1, KL + 2], BF16, tag="p1t", name="p1t")
                P2T = npool.tile([1, KL + 2], BF16, tag="p2t", name="p2t")

                def p_chunk(off, sz):
                    pps = ps_p.tile([3, 512], F32, tag="pps")
                    for c in range(NC_C):
                        nc.tensor.matmul(
                            pps[0:3, 0:sz],
                            bkp_sb[:, c * KW : (c + 1) * KW],
                            kT[c][:, off : off + sz],
                            start=(c == 0),
                            stop=(c == NC_C - 1),
                        )
                    nc.scalar.activation(
                        P_sb[0:3, off : off + sz], pps[0:3, 0:sz], Identity
                    )

                p_chunk(0, 512)
                p_chunk(512, 2)

                # ---- stage 1: N'[s][p, j] = sum_{c,t} Wk[3c+t, s]*k_pad[j+t, c] + Wb[s]
                N = [npool.tile([128, KL], BF16, tag=f"N{s}", name=f"N{s}")
                     for s in range(NC_S)]
                for jh in range(NJH):
                    if jh == 1:
                        p_chunk(514, 510)
                        p_chunk(1024, 2)
                        nc.sync.dma_start(P1T[0:1, :], P_sb[1:2, :])
                        nc.sync.dma_start(P2T[0:1, :], P_sb[2:3, :])
                    for s in range(NC_S):
                        nps = ps_n.tile([128, 512], F32, tag="nps")
                        first = True
                        for c in range(NC_C):
                            for t in range(KW):
                                nc.tensor.matmul(
                                    nps[:],
                                    wk_sb[c][:, t * QS + s * 128 : t * QS + (s + 1) * 128],
                                    kT[c][:, jh * 512 + t : jh * 512 + t + 512],
                                    start=first,
                                    stop=(c == NC_C - 1 and t == KW - 1),
                                )
                                first = False
                        nc.scalar.activation(
                            N[s][:, jh * 512 : (jh + 1) * 512], nps[:], Identity,
                            bias=cons_sb[:, s : s + 1],
                        )

                # ---- r'[p, j] = sum_t P[t, j+t] + bb + bias_b  (rank-1 shift-sum,
                #      all partitions identical)
                r_sb = npool.tile([128, KL], F32, tag="rsb", name="rsb")
                for jh in range(NJH):
                    rps = ps_r.tile([128, 512], F32, tag="rps")
                    for t, row in ((0, P_sb), (1, P1T), (2, P2T)):
                        nc.tensor.matmul(
                            rps[:],
                            ones_sb[:],
                            row[0:1, jh * 512 + t : jh * 512 + t + 512],
                            start=(t == 0),
                            stop=(t == KW - 1),
                        )
                    nc.scalar.activation(
                        r_sb[:, jh * 512 : (jh + 1) * 512], rps[:], Identity,
                        bias=cons_sb[:, 4:5],
                    )

                # ---- stage 2 + fused epilogue: out = qT.T @ N' + r'
                for i in range(NI):
                    out_sb = opool.tile([128, KL], BF16, tag="osb")
                    for jh in range(NJH):
                        sps = ps_s.tile([128, 512], F32, tag="sps")
                        for c in range(NC_S):
                            nc.tensor.matmul(
                                sps[:],
                                qT[c][:, i * 128 : (i + 1) * 128],
                                N[c][:, jh * 512 : (jh + 1) * 512],
                                start=(c == 0),
                                stop=(c == NC_S - 1),
                            )
                        nc.vector.tensor_tensor(
                            out_sb[:, jh * 512 : (jh + 1) * 512],
                            sps[:],
                            r_sb[:, jh * 512 : (jh + 1) * 512],
                            ADD,
                        )
                    # spread the last batch's final tiles across idle rings so
                    # the end-of-kernel DMA drain happens in parallel (earlier
                    # batches keep sync/vector free for the next batch's loads)
                    last_b = b == B_LOC - 1
                    if last_b and i == NI - 2:
                        eng = nc.sync
                    else:
                        eng = nc.gpsimd if i % 2 == 0 else nc.scalar
                    eng.dma_start(out_d[b, i * 128 : (i + 1) * 128, :], out_sb[:])
    nc.finalize()
    return nc


def _get_nc():
    if "nc" not in _NC_CACHE:
        _NC_CACHE["nc"] = _build()
    return _NC_CACHE["nc"]


def _prep_inputs(q, k, Wk, bk, Wb, bb, bias_b):
    q = np.asarray(q, np.float32)
    k = np.asarray(k, np.float32)
    Wk = np.asarray(Wk, np.float32)
    bk = np.asarray(bk, np.float32)
    Wb = np.asarray(Wb, np.float32)
    bb = np.asarray(bb, np.float32)
    bias_b = np.asarray(bias_b, np.float32)

    qT = q.transpose(0, 2, 1).astype(BF)                    # [B, QS, QL]
    kT = np.zeros((B, KS, KL + 2), dtype=BF)
    kT[:, :, 1 : KL + 1] = k.transpose(0, 2, 1).astype(BF)  # zero halo cols 0, KL+1
    # wk[c][p, t*QS + s] = Wk[3*(c*128+p) + t, s]
    wk = np.ascontiguousarray(Wk.reshape(NC_C, 128, KW, QS)).reshape(
        NC_C, 128, KW * QS).astype(BF)
    # bkp[p, c*3 + t] = bk[3*(c*128+p) + t]
    bkp = np.ascontiguousarray(
        bk.reshape(NC_C, 128, KW).transpose(1, 0, 2)).reshape(
        128, NC_C * KW).astype(BF)
    cons = np.zeros((128, 8), np.float32)
    cons[:, 0:NC_S] = Wb.reshape(NC_S, 128).T               # Wb^T per s-chunk
    cons[:, 4] = bb[0] + bias_b[0]

    in_maps = []
    for core in range(NCORES):
        lo = core * B_LOC
        in_maps.append({
            "qT": np.ascontiguousarray(qT[lo : lo + B_LOC]),
            "kT": kT[lo : lo + B_LOC],
            "wk": wk,
            "bkp": bkp,
            "cons": cons,
        })
    return in_maps


def kernel(q, k, Wk, bk, Wb, bb, bias_b):
    nc = _get_nc()
    in_maps = _prep_inputs(q, k, Wk, bk, Wb, bb, bias_b)
    res = run_bass_kernel_spmd(nc, in_maps, list(range(NCORES)))
    out = np.concatenate([np.asarray(res.results[c]["out"]) for c in range(NCORES)],
                         axis=0)
    return out.astype(np.float32)
